# revision 27
# baseline (speedup 1.0000x reference)
"""Multi-head attention (b=2, n=2048, dim=1024, 16 heads x 64) on 8 TRN2 NeuronCores.

Sharding: core c handles batch c//4 and heads 4*(c%4) .. 4*(c%4)+3
(data parallel over batch x 4-way head/tensor parallel). w_qkv is
column-sharded by head; w_out is column-sharded: each core computes a
256-column slice of the output after AllGathers of the attention outputs
within its 4-core batch group (no all-reduce needed).

Device layout is feature-major ("K-major"): x arrives pre-transposed
[dim, n] in bf16; Q^T/K^T are produced feature-major (weight-stationary
matmul order to minimize LDWEIGHTS) and V token-major; attention scores
are computed transposed (dotsT[k, q]); softmax sums come from an
augmented ones-column in the V matmul; softmax exp runs on the scalar
engine with the 1/sqrt(d) scale folded in. 1/Z uses the custom-DVE
reciprocal_approx_fast; Z^-1 is broadcast across partitions by a rank-1
PE matmul. The AllGather is split per (head-pair, 512-token quarter) —
8 collectives fired as each attention block finishes (chain compressed
to kc 0-4, AG at kc==5) — and the output projection for each gathered
piece is interleaved into the attention steady state. The tail is a
single tightly-chained AG + final projection piece. The final output
is produced transposed [cols, n]; the host transposes back.
"""

import sys

sys.path.insert(0, "/opt/trn_rl_repo")

import ml_dtypes
import numpy as np

import concourse.bass as bass  # noqa: F401  (engine types)
import concourse.tile as tile
from concourse import bacc, mybir
from concourse.bass_utils import run_bass_kernel_spmd

F32 = mybir.dt.float32
F32R = mybir.dt.float32r
BF16 = mybir.dt.bfloat16
NP_BF16 = np.dtype(ml_dtypes.bfloat16)

# Problem constants
B, N, DIM = 2, 2048, 1024
HEADS, DH = 16, 64
INNER = HEADS * DH
SCALE = DH ** -0.5
CORES = 8
GROUP_SIZE = 4
REPLICA_GROUPS = [[0, 1, 2, 3], [4, 5, 6, 7]]
HPC = 4  # heads per core
CS = HPC * DH  # 256 per-core feature columns

KC = DIM // 128  # 8 contraction chunks for dim
TT = N // 128  # 16 token tiles
QB = N // 512  # 4 q blocks
NKC = N // 128  # 16 key chunks
NBLK = 2 * QB  # 8 attention blocks: (head pair, 512-query quarter)


def build_nc():
    nc = bacc.Bacc("TRN2", target_bir_lowering=False, debug=False, num_devices=CORES)
    xt = nc.dram_tensor("xt", [DIM, N], BF16, kind="ExternalInput").ap()
    wq = nc.dram_tensor("wq", [DIM, CS], BF16, kind="ExternalInput").ap()
    wk = nc.dram_tensor("wk", [DIM, CS], BF16, kind="ExternalInput").ap()
    wv = nc.dram_tensor("wv", [DIM, CS], BF16, kind="ExternalInput").ap()
    wo = nc.dram_tensor("wo", [INNER, CS], BF16, kind="ExternalInput").ap()
    bo = nc.dram_tensor("bo", [CS], F32, kind="ExternalInput").ap()
    y = nc.dram_tensor("y", [CS, N], F32, kind="ExternalOutput").ap()  # y^T

    cc_in = [nc.dram_tensor(f"cc_in{b}", [128, 512], BF16) for b in range(NBLK)]
    cc_out = [
        nc.dram_tensor(f"cc_out{b}", [GROUP_SIZE * 128, 512], BF16)
        for b in range(NBLK)
    ]
    with tile.TileContext(nc) as tc:
        with (
            tc.tile_pool(name="big", bufs=2) as big,  # xt, then the AG results
            tc.tile_pool(name="sb", bufs=1) as sb,
            tc.tile_pool(name="expp", bufs=4) as expp,
            tc.tile_pool(name="yout", bufs=3) as yout,
            tc.tile_pool(name="norm", bufs=8) as normp,
            tc.tile_pool(name="psd", bufs=2, space="PSUM") as psd,  # 4 banks
            tc.tile_pool(name="pso", bufs=2, space="PSUM") as pso,  # 2 banks
            # shared 2-slot ring for Z-broadcast + out-proj accumulators
            tc.tile_pool(name="psx", bufs=2, space="PSUM") as psx,  # 2 banks
        ):
            # ---- load inputs -------------------------------------------------
            xt_sb = big.tile([128, KC, N], BF16, tag="bigbuf")
            wq_sb = sb.tile([128, KC, CS], BF16)
            wk_sb = sb.tile([128, KC, CS], BF16)
            wv_sb = sb.tile([128, KC, CS], BF16)
            wo_sb = sb.tile([128, KC, CS], BF16)
            nc.sync.dma_start(out=wq_sb, in_=wq.rearrange("(c p) n -> p c n", p=128))
            nc.sync.dma_start(out=wk_sb, in_=wk.rearrange("(c p) n -> p c n", p=128))
            xt_r = xt.rearrange("(c p) n -> p c n", p=128)
            for c in range(KC):
                for qb in range(QB):
                    sl = slice(qb * 512, (qb + 1) * 512)
                    eng = nc.sync if (c * QB + qb) % 2 == 0 else nc.gpsimd
                    eng.dma_start(out=xt_sb[:, c, sl], in_=xt_r[:, c, sl])
            nc.sync.dma_start(out=wv_sb, in_=wv.rearrange("(c p) n -> p c n", p=128))
            nc.sync.dma_start(out=wo_sb, in_=wo.rearrange("(c p) n -> p c n", p=128))

            # bias, transposed layout: partition = column-within-block
            bias_sb = sb.tile([128, 2], F32)
            nc.sync.dma_start(out=bias_sb, in_=bo.rearrange("(cb p) -> p cb", p=128))

            ones_f = sb.tile([128, TT], F32)
            nc.vector.memset(ones_f, 1.0)

            # ---- QKV projection ---------------------------------------------
            qt_sb = sb.tile([128, 2, N], BF16)
            kt_sb = sb.tile([128, 2, N], BF16)
            vaug = sb.tile([128, TT, HPC, DH + 1], BF16)
            with nc.allow_low_precision(reason="bf16 ones column"):
                for h in range(HPC):
                    nc.vector.tensor_copy(vaug[:, :, h, DH], ones_f)

            def qk_pass(m, dst, w_sb):
                # weight-stationary: one LDWEIGHTS per (m, c); 4 query blocks
                # stream through the same loaded weights.
                pa = psd.tile([128, 2, 512], F32, name="psd")
                pb = psd.tile([128, 2, 512], F32, name="psd")
                accs = [pa[:, 0, :], pa[:, 1, :], pb[:, 0, :], pb[:, 1, :]]
                for c in range(KC):
                    for qb in range(QB):
                        nc.tensor.matmul(
                            accs[qb],
                            lhsT=w_sb[:, c, m * 128 : (m + 1) * 128],
                            rhs=xt_sb[:, c, qb * 512 : (qb + 1) * 512],
                            start=(c == 0),
                            stop=(c == KC - 1),
                        )
                with nc.allow_low_precision(reason="bf16 attention"):
                    for qb in range(QB):
                        nc.vector.tensor_copy(
                            dst[:, m, qb * 512 : (qb + 1) * 512], accs[qb]
                        )

            # hp0's q/k only — hp1 is produced inside the attention steady
            # state (task queue below), so attention starts ~30us earlier.
            qk_pass(0, qt_sb, wq_sb)
            qk_pass(0, kt_sb, wk_sb)

            def emit_v(t):
                # psx ring (not psd): keeps the dots double-buffer intact
                # when v-production interleaves with attention block 0.
                ps = psx.tile([128, 512], F32, name="psv", tag="psx")
                acc = ps[:, 0:CS]
                for c in range(KC):
                    nc.tensor.matmul(
                        acc,
                        lhsT=xt_sb[:, c, t * 128 : (t + 1) * 128],
                        rhs=wv_sb[:, c, :],
                        start=(c == 0),
                        stop=(c == KC - 1),
                    )
                with nc.allow_low_precision(reason="bf16 attention"):
                    nc.vector.tensor_copy(
                        vaug[:, t, :, 0:DH],
                        acc.rearrange("p (h d) -> p h d", d=DH),
                    )

            for t in range(8):
                emit_v(t)

            # hp1 q/k as interleaved tasks: per (dst, qb) one psx
            # accumulator, 8 contraction chunks, then a copy. Popped from
            # the task queue during blocks 1-2 (2 tasks/step at kc>=6).
            def qk1_tasks():
                tasks = []

                def mk_mm(dst, w_sb, qb, c0, st):
                    def f():
                        if c0 == 0:
                            st["ps"] = psx.tile(
                                [128, 512], F32, name="qk1", tag="psx"
                            )
                        for c in range(c0, c0 + 4):
                            nc.tensor.matmul(
                                st["ps"],
                                lhsT=w_sb[:, c, 128:256],
                                rhs=xt_sb[:, c, qb * 512 : (qb + 1) * 512],
                                start=(c == 0),
                                stop=(c == KC - 1),
                            )

                    return f

                def mk_cp(dst, qb, st):
                    def f():
                        with nc.allow_low_precision(reason="bf16 attention"):
                            nc.vector.tensor_copy(
                                dst[:, 1, qb * 512 : (qb + 1) * 512], st["ps"]
                            )

                    return f

                for dst, w_sb in ((qt_sb, wq_sb), (kt_sb, wk_sb)):
                    for qb in range(QB):
                        st = {}
                        tasks += [
                            mk_mm(dst, w_sb, qb, 0, st),
                            mk_mm(dst, w_sb, qb, 4, st),
                            mk_cp(dst, qb, st),
                        ]
                return tasks

            # ---- attention + per-block AllGather + interleaved out-proj -----
            outt_sb = sb.tile([128, 2, N], BF16)
            ag_all = big.tile(
                [128, 2, QB, GROUP_SIZE, 512], BF16, tag="bigbuf"
            )  # [p, hp, qb, src_core, tok]

            def emit_dots(blk, kc):
                hp, qb = divmod(blk, 4)
                ps = psd.tile([128, 2, 512], F32, name="psd")
                for hh in range(2):
                    base = hh * DH
                    nc.tensor.matmul(
                        ps[:, hh, :],
                        lhsT=kt_sb[base : base + DH, hp, kc * 128 : (kc + 1) * 128],
                        rhs=qt_sb[base : base + DH, hp, qb * 512 : (qb + 1) * 512],
                        start=True,
                        stop=True,
                        tile_position=(base, 0),
                    )
                ex = expp.tile([128, 2, 512], BF16, name="expT")
                nc.scalar.activation(
                    out=ex, in_=ps, func=mybir.ActivationFunctionType.Exp, scale=SCALE
                )
                return ex

            def emit_attv(blk, kc, ex, po):
                hp = blk // 4
                for hh in range(2):
                    nc.tensor.matmul(
                        po[hh],
                        lhsT=vaug[:, kc, hp * 2 + hh, :],
                        rhs=ex[:, hh, :],
                        start=(kc == 0),
                        stop=(kc == NKC - 1),
                    )

            def emit_posb(po):
                # drain PSUM accumulators to SBUF right away to free the ring
                po_sbs = []
                for hh in range(2):
                    po_sb = normp.tile([DH + 1, 512], F32, name="po_sb")
                    nc.vector.tensor_copy(po_sb, po[hh])
                    po_sbs.append(po_sb)
                return po_sbs

            def emit_zbc(po_sbs, hh):
                # broadcast the raw Z row (po_sb partition 64) across 64
                # partitions with a stride-0 DMA. DMA is partition-agnostic,
                # unlike the custom-DVE reciprocal which silently misreads
                # partition-offset inputs. gpsimd queue: its other work
                # (cc_in spill + collective trigger) returns immediately,
                # so no head-of-line blocking (sync's queue stalls on
                # ag_all DMAs awaiting collectives).
                zbr = normp.tile([DH, 512], F32, name="zbr")
                nc.gpsimd.dma_start(
                    out=zbr,
                    in_=po_sbs[hh][DH : DH + 1, :]
                    .rearrange("p (o n) -> p o n", o=1)
                    .broadcast_to([1, DH, 512]),
                )
                return zbr

            def emit_recip(zbr):
                # 1/Z on partition-0-based [64,512] (custom-DVE fast recip)
                zbi = normp.tile([DH, 512], F32, name="zbi")
                nc.vector.reciprocal_approx_fast(out=zbi, in_=zbr)
                return zbi

            def emit_mul(blk, po_sbs, zbi, hh):
                hp, qb = divmod(blk, 4)
                base = hh * DH
                with nc.allow_low_precision(reason="bf16 attention out"):
                    nc.vector.tensor_mul(
                        outt_sb[base : base + DH, hp, qb * 512 : (qb + 1) * 512],
                        po_sbs[hh][0:DH, :],
                        zbi,
                    )

            def emit_ag(blk):
                hp, qb = divmod(blk, 4)
                sl = slice(qb * 512, (qb + 1) * 512)
                nc.gpsimd.dma_start(out=cc_in[blk].ap(), in_=outt_sb[:, hp, sl])
                nc.gpsimd.collective_compute(
                    "AllGather",
                    mybir.AluOpType.bypass,
                    ins=[cc_in[blk].ap().opt()],
                    outs=[cc_out[blk].ap().opt()],
                    replica_groups=REPLICA_GROUPS,
                )
                nc.sync.dma_start(
                    out=ag_all[:, hp, qb, :, :],
                    in_=cc_out[blk].ap().rearrange("(c p) n -> p c n", p=128),
                )

            # --- interleaved output projection -------------------------------
            # piece qb: y^T[:, qb] = sum_hp wo[hp]^T @ ag[hp][qb] + bias.
            # Blocks alternate head pairs, so both AGs of a token quarter
            # land early and each piece is a single 8-chunk PSUM
            # accumulation per column block — no SBUF staging.
            def proj_tasks(qb):
                tasks = []
                ps_ref = {}

                def mk_mm(cb, hp, c0):
                    def f():
                        if hp == 0 and c0 == 0:
                            ps_ref[cb] = psx.tile(
                                [128, 512], F32, name="psy", tag="psx"
                            )
                        for c in range(c0, c0 + 2):
                            nc.tensor.matmul(
                                ps_ref[cb],
                                lhsT=wo_sb[:, hp * 4 + c, cb * 128 : (cb + 1) * 128],
                                rhs=ag_all[:, hp, qb, c, :],
                                start=(hp == 0 and c == 0),
                                stop=(hp == 1 and c == 3),
                            )

                    return f

                def mk_fin(cb):
                    def f():
                        qsl = slice(qb * 512, (qb + 1) * 512)
                        y_sb = yout.tile([128, 512], F32, name="y_sb")
                        nc.vector.tensor_scalar_add(
                            out=y_sb,
                            in0=ps_ref[cb],
                            scalar1=bias_sb[:, cb : cb + 1],
                        )
                        nc.sync.dma_start(
                            out=y[cb * 128 : (cb + 1) * 128, qsl], in_=y_sb
                        )

                    return f

                for cb in range(2):
                    for hp in range(2):
                        tasks.append(mk_mm(cb, hp, 0))
                        tasks.append(mk_mm(cb, hp, 2))
                    tasks.append(mk_fin(cb))
                return tasks

            # hp-major block order: piece qb needs AG(blk qb) (hp0, fired
            # block qb+1) and AG(blk 4+qb) (hp1, fired block 5+qb at kc==4).
            # Pieces 2 and 3 land in the tail, overlapping the final AG.
            proj_sched = {6: [0], 7: [1]}


            # one continuous software-pipelined stream over all 8 blocks:
            # attV lags dots/exp by one step; po drains to SBUF right after a
            # block's last attV; 1/Z, broadcast, and mul stages run over the
            # next block's early steps; the AllGather fires at kc==4.
            pend_attv = None  # (blk, kc, ex)
            po_cur = None
            posb_prev = None  # po_sbs of previous block
            zbi_prev = [None, None]
            task_q = []
            for step in range(NBLK * NKC):
                blk, kc = divmod(step, NKC)
                if kc == 0:
                    po_prev = po_cur
                    po_cur = [
                        pso.tile([DH + 1, 512], F32, name="ps_o") for _ in range(2)
                    ]
                    if blk == 1:
                        task_q += qk1_tasks()
                    task_q += [
                        t for q in proj_sched.get(blk, []) for t in proj_tasks(q)
                    ]
                ex = emit_dots(blk, kc)
                if pend_attv is not None:
                    pblk, pkc, pex = pend_attv
                    emit_attv(pblk, pkc, pex, po_cur if pblk == blk else po_prev)
                    if pkc == NKC - 1:
                        posb_cur = emit_posb(po_prev)
                pend_attv = (blk, kc, ex)
                if blk > 0:
                    if kc == 0:
                        posb_prev = posb_cur
                    elif kc == 1:
                        zbr_prev = [
                            emit_zbc(posb_prev, 0),
                            emit_zbc(posb_prev, 1),
                        ]
                    elif kc == 2:
                        zbi_prev[0] = emit_recip(zbr_prev[0])
                        zbi_prev[1] = emit_recip(zbr_prev[1])
                    elif kc == 3:
                        emit_mul(blk - 1, posb_prev, zbi_prev[0], 0)
                        emit_mul(blk - 1, posb_prev, zbi_prev[1], 1)
                    elif kc == 4:
                        emit_ag(blk - 1)
                if blk == 0 and kc < TT - 8:
                    emit_v(kc + 8)
                if kc >= 6:
                    for _ in range(2):
                        if task_q:
                            task_q.pop(0)()
            # drain: the last block's normalization chain runs immediately
            # after its final attV so the last AllGather fires ASAP; proj
            # pieces 2 and 3 run after, overlapping the AG latencies.
            for t in task_q:
                t()
            pblk, pkc, pex = pend_attv
            emit_attv(pblk, pkc, pex, po_cur)
            po_sbs = emit_posb(po_cur)
            zbr0 = emit_zbc(po_sbs, 0)
            zbr1 = emit_zbc(po_sbs, 1)
            zbi0 = emit_recip(zbr0)
            zbi1 = emit_recip(zbr1)
            emit_mul(NBLK - 1, po_sbs, zbi0, 0)
            emit_mul(NBLK - 1, po_sbs, zbi1, 1)
            emit_ag(NBLK - 1)
            for t in proj_tasks(2):
                t()
            for t in proj_tasks(3):
                t()

    nc.compile()
    return nc


_NC_CACHE = None


def _get_nc():
    global _NC_CACHE
    if _NC_CACHE is None:
        _NC_CACHE = build_nc()
    return _NC_CACHE


def _wo_perm(w_out):
    # chunk order [AG-hp0: r0..r3 -> w_out rows 256r..256r+128,
    #              AG-hp1: r0..r3 -> w_out rows 256r+128..256r+256]
    blocks = [w_out[256 * r : 256 * r + 128] for r in range(4)]
    blocks += [w_out[256 * r + 128 : 256 * r + 256] for r in range(4)]
    return np.concatenate(blocks, axis=0)


def _make_in_maps(x, w_qkv, w_out, b_out):
    wop = _wo_perm(w_out)
    in_maps = []
    for c in range(CORES):
        bi = c // GROUP_SIZE
        g = c % GROUP_SIZE
        cols = slice(g * CS, (g + 1) * CS)
        in_maps.append(
            {
                "xt": np.ascontiguousarray(x[bi].T).astype(NP_BF16),
                "wq": np.ascontiguousarray(w_qkv[:, cols]).astype(NP_BF16),
                "wk": np.ascontiguousarray(w_qkv[:, INNER:][:, cols]).astype(NP_BF16),
                "wv": np.ascontiguousarray(w_qkv[:, 2 * INNER:][:, cols]).astype(
                    NP_BF16
                ),
                "wo": np.ascontiguousarray(wop[:, cols]).astype(NP_BF16),
                "bo": np.ascontiguousarray(b_out[cols]),
            }
        )
    return in_maps


def _assemble(results):
    out = np.empty((B, N, DIM), dtype=np.float32)
    for c in range(CORES):
        bi = c // GROUP_SIZE
        g = c % GROUP_SIZE
        out[bi, :, g * CS : (g + 1) * CS] = results[c]["y"].T
    return out


def kernel(x, w_qkv, w_out, b_out, _trace=False, _trace_kwargs=None):
    x = np.asarray(x, dtype=np.float32)
    w_qkv = np.asarray(w_qkv, dtype=np.float32)
    w_out = np.asarray(w_out, dtype=np.float32)
    b_out = np.asarray(b_out, dtype=np.float32)
    nc = _get_nc()
    in_maps = _make_in_maps(x, w_qkv, w_out, b_out)
    res = run_bass_kernel_spmd(
        nc,
        in_maps,
        core_ids=list(range(CORES)),
        trace=_trace,
        **(_trace_kwargs or {}),
    )
    out = _assemble(res.results)
    if _trace:
        return out, res
    return out



# revision 34
# speedup vs baseline: 1.1336x; 1.1336x over previous
"""Multi-head attention (b=2, n=2048, dim=1024, 16 heads x 64) on 8 TRN2 NeuronCores.

Sharding: core c handles batch c//4 and heads 4*(c%4) .. 4*(c%4)+3
(data parallel over batch x 4-way head/tensor parallel). w_qkv is
column-sharded by head; w_out is column-sharded: each core computes a
256-column slice of the output after AllGathers of the attention outputs
within its 4-core batch group (no all-reduce needed).

Device layout is feature-major ("K-major"): x arrives pre-transposed
[dim, n] in bf16; Q^T/K^T are produced feature-major (weight-stationary
matmul order to minimize LDWEIGHTS) and V token-major; attention scores
are computed transposed (dotsT[k, q]); softmax sums come from an
augmented ones-column in the V matmul; softmax exp runs on the scalar
engine with the 1/sqrt(d) scale folded in. 1/Z uses the custom-DVE
reciprocal_approx_fast; Z^-1 is broadcast across partitions by a rank-1
PE matmul. The AllGather is split per (head-pair, 512-token quarter) —
8 collectives fired as each attention block finishes (chain compressed
to kc 0-4, AG at kc==5) — and the output projection for each gathered
piece is interleaved into the attention steady state. The tail is a
single tightly-chained AG + final projection piece. The final output
is produced transposed [cols, n]; the host transposes back.
"""

import sys

sys.path.insert(0, "/opt/trn_rl_repo")

import ml_dtypes
import numpy as np

import concourse.bass as bass  # noqa: F401  (engine types)
import concourse.tile as tile
from concourse import bacc, mybir
from concourse.bass_utils import run_bass_kernel_spmd

F32 = mybir.dt.float32
F32R = mybir.dt.float32r
BF16 = mybir.dt.bfloat16
NP_BF16 = np.dtype(ml_dtypes.bfloat16)

# Problem constants
B, N, DIM = 2, 2048, 1024
HEADS, DH = 16, 64
INNER = HEADS * DH
SCALE = DH ** -0.5
CORES = 8
GROUP_SIZE = 4
REPLICA_GROUPS = [[0, 1, 2, 3], [4, 5, 6, 7]]
HPC = 4  # heads per core
CS = HPC * DH  # 256 per-core feature columns

KC = DIM // 128  # 8 contraction chunks for dim
TT = N // 128  # 16 token tiles
QB = N // 512  # 4 q blocks
NKC = N // 128  # 16 key chunks
NBLK = 2 * QB  # 8 attention blocks: (head pair, 512-query quarter)


def build_nc():
    nc = bacc.Bacc("TRN2", target_bir_lowering=False, debug=False, num_devices=CORES)
    xt = nc.dram_tensor("xt", [DIM, N], BF16, kind="ExternalInput").ap()
    wq = nc.dram_tensor("wq", [DIM, CS], BF16, kind="ExternalInput").ap()
    wk = nc.dram_tensor("wk", [DIM, CS], BF16, kind="ExternalInput").ap()
    wv = nc.dram_tensor("wv", [DIM, CS], BF16, kind="ExternalInput").ap()
    wo = nc.dram_tensor("wo", [INNER, CS], BF16, kind="ExternalInput").ap()
    bo = nc.dram_tensor("bo", [CS], F32, kind="ExternalInput").ap()
    y = nc.dram_tensor("y", [CS, N], F32, kind="ExternalOutput").ap()  # y^T

    cc_in = [nc.dram_tensor(f"cc_in{b}", [128, 512], BF16) for b in range(NBLK)]
    cc_out = [
        nc.dram_tensor(f"cc_out{b}", [GROUP_SIZE * 128, 512], BF16)
        for b in range(NBLK)
    ]
    with tile.TileContext(nc) as tc:
        with (
            tc.tile_pool(name="big", bufs=2) as big,  # xt, then the AG results
            tc.tile_pool(name="sb", bufs=1) as sb,
            tc.tile_pool(name="expp", bufs=4) as expp,
            tc.tile_pool(name="yout", bufs=3) as yout,
            tc.tile_pool(name="norm", bufs=8) as normp,
            tc.tile_pool(name="zv", bufs=4) as zvp,
            tc.tile_pool(name="psd", bufs=2, space="PSUM") as psd,  # 4 banks
            tc.tile_pool(name="pso", bufs=2, space="PSUM") as pso,  # 2 banks
            # shared 2-slot ring for Z-broadcast + out-proj accumulators
            tc.tile_pool(name="psx", bufs=2, space="PSUM") as psx,  # 2 banks
        ):
            # ---- load inputs -------------------------------------------------
            xt_sb = big.tile([128, KC, N], BF16, tag="bigbuf")
            wq_sb = sb.tile([128, KC, CS], BF16)
            wk_sb = sb.tile([128, KC, CS], BF16)
            wv_sb = sb.tile([128, KC, CS], BF16)
            wo_sb = sb.tile([128, KC, CS], BF16)
            nc.sync.dma_start(out=wq_sb, in_=wq.rearrange("(c p) n -> p c n", p=128))
            nc.sync.dma_start(out=wk_sb, in_=wk.rearrange("(c p) n -> p c n", p=128))
            xt_r = xt.rearrange("(c p) n -> p c n", p=128)
            for c in range(KC):
                for qb in range(QB):
                    sl = slice(qb * 512, (qb + 1) * 512)
                    eng = nc.sync if (c * QB + qb) % 2 == 0 else nc.gpsimd
                    eng.dma_start(out=xt_sb[:, c, sl], in_=xt_r[:, c, sl])
            nc.sync.dma_start(out=wv_sb, in_=wv.rearrange("(c p) n -> p c n", p=128))
            nc.sync.dma_start(out=wo_sb, in_=wo.rearrange("(c p) n -> p c n", p=128))

            # bias, transposed layout: partition = column-within-block
            bias_sb = sb.tile([128, 2], F32)
            nc.sync.dma_start(out=bias_sb, in_=bo.rearrange("(cb p) -> p cb", p=128))

            ones_f = sb.tile([128, TT], F32)
            nc.vector.memset(ones_f, 1.0)
            ones_r = sb.tile([1, DH], F32R)
            nc.vector.tensor_copy(ones_r, ones_f[0:1, 0:1].broadcast_to([1, DH]))
            ones_q = sb.tile([1, 512], F32R)
            nc.vector.tensor_copy(ones_q, ones_f[0:1, 0:1].broadcast_to([1, 512]))

            # HAM warm-up: ~4us of back-to-back rank-1 matmuls while the
            # input DMAs stream in, so the PE clock is at 8/8 (2.4GHz)
            # when the QKV pass starts instead of ramping mid-pass.
            warm_ps = psx.tile([DH, 512], F32, name="warm", tag="psx")
            for _ in range(10):
                nc.tensor.matmul(
                    warm_ps, lhsT=ones_r, rhs=ones_q, start=True, stop=True
                )

            # ---- QKV projection ---------------------------------------------
            qt_sb = sb.tile([128, 2, N], BF16)
            kt_sb = sb.tile([128, 2, N], BF16)
            vaug = sb.tile([128, TT, HPC, DH + 1], BF16)
            with nc.allow_low_precision(reason="bf16 ones column"):
                for h in range(HPC):
                    nc.vector.tensor_copy(vaug[:, :, h, DH], ones_f)

            def qk_pass(m, dst, w_sb):
                # weight-stationary: one LDWEIGHTS per (m, c); 4 query blocks
                # stream through the same loaded weights.
                pa = psd.tile([128, 2, 512], F32, name="psd")
                pb = psd.tile([128, 2, 512], F32, name="psd")
                accs = [pa[:, 0, :], pa[:, 1, :], pb[:, 0, :], pb[:, 1, :]]
                for c in range(KC):
                    for qb in range(QB):
                        nc.tensor.matmul(
                            accs[qb],
                            lhsT=w_sb[:, c, m * 128 : (m + 1) * 128],
                            rhs=xt_sb[:, c, qb * 512 : (qb + 1) * 512],
                            start=(c == 0),
                            stop=(c == KC - 1),
                        )
                with nc.allow_low_precision(reason="bf16 attention"):
                    for qb in range(QB):
                        nc.vector.tensor_copy(
                            dst[:, m, qb * 512 : (qb + 1) * 512], accs[qb]
                        )

            # hp0's q/k only — hp1 is produced inside the attention steady
            # state (task queue below), so attention starts ~30us earlier.
            qk_pass(0, qt_sb, wq_sb)
            qk_pass(0, kt_sb, wk_sb)

            def emit_v(t):
                # psx ring (not psd): keeps the dots double-buffer intact
                # when v-production interleaves with attention block 0.
                ps = psx.tile([128, 512], F32, name="psv", tag="psx")
                acc = ps[:, 0:CS]
                for c in range(KC):
                    nc.tensor.matmul(
                        acc,
                        lhsT=xt_sb[:, c, t * 128 : (t + 1) * 128],
                        rhs=wv_sb[:, c, :],
                        start=(c == 0),
                        stop=(c == KC - 1),
                    )
                with nc.allow_low_precision(reason="bf16 attention"):
                    nc.vector.tensor_copy(
                        vaug[:, t, :, 0:DH],
                        acc.rearrange("p (h d) -> p h d", d=DH),
                    )

            for t in range(8):
                emit_v(t)

            # hp1 q/k as interleaved tasks: per (dst, qb) one psx
            # accumulator, 8 contraction chunks, then a copy. 5 tasks per
            # unit so the 20 pops/block (2/step at kc>=6) align exactly
            # with unit boundaries — a unit never holds its psx slot
            # across a block boundary (the zb broadcasts need both slots
            # at kc==2).
            def qk1_tasks():
                tasks = []

                def mk_mm(dst, w_sb, qb, c0, st):
                    def f():
                        if c0 == 0:
                            st["ps"] = psx.tile(
                                [128, 512], F32, name="qk1", tag="psx"
                            )
                        for c in range(c0, c0 + 2):
                            nc.tensor.matmul(
                                st["ps"],
                                lhsT=w_sb[:, c, 128:256],
                                rhs=xt_sb[:, c, qb * 512 : (qb + 1) * 512],
                                start=(c == 0),
                                stop=(c == KC - 1),
                            )

                    return f

                def mk_cp(dst, qb, st):
                    def f():
                        with nc.allow_low_precision(reason="bf16 attention"):
                            nc.vector.tensor_copy(
                                dst[:, 1, qb * 512 : (qb + 1) * 512], st["ps"]
                            )

                    return f

                for dst, w_sb in ((qt_sb, wq_sb), (kt_sb, wk_sb)):
                    for qb in range(QB):
                        st = {}
                        tasks += [
                            mk_mm(dst, w_sb, qb, 0, st),
                            mk_mm(dst, w_sb, qb, 2, st),
                            mk_mm(dst, w_sb, qb, 4, st),
                            mk_mm(dst, w_sb, qb, 6, st),
                            mk_cp(dst, qb, st),
                        ]
                return tasks

            # ---- attention + per-block AllGather + interleaved out-proj -----
            outt_sb = sb.tile([128, 2, N], BF16)
            ag_all = big.tile(
                [128, 2, QB, GROUP_SIZE, 512], BF16, tag="bigbuf"
            )  # [p, hp, qb, src_core, tok]

            def emit_dots(blk, kc):
                hp, qb = divmod(blk, 4)
                ps = psd.tile([128, 2, 512], F32, name="psd")
                for hh in range(2):
                    base = hh * DH
                    nc.tensor.matmul(
                        ps[:, hh, :],
                        lhsT=kt_sb[base : base + DH, hp, kc * 128 : (kc + 1) * 128],
                        rhs=qt_sb[base : base + DH, hp, qb * 512 : (qb + 1) * 512],
                        start=True,
                        stop=True,
                        tile_position=(base, 0),
                    )
                ex = expp.tile([128, 2, 512], BF16, name="expT")
                nc.scalar.activation(
                    out=ex, in_=ps, func=mybir.ActivationFunctionType.Exp, scale=SCALE
                )
                return ex

            def emit_attv(blk, kc, ex, po):
                hp = blk // 4
                for hh in range(2):
                    nc.tensor.matmul(
                        po[hh],
                        lhsT=vaug[:, kc, hp * 2 + hh, :],
                        rhs=ex[:, hh, :],
                        start=(kc == 0),
                        stop=(kc == NKC - 1),
                    )

            def emit_posb(po):
                # drain PSUM accumulators to SBUF right away to free the ring
                po_sbs = []
                for hh in range(2):
                    po_sb = normp.tile([DH + 1, 512], F32, name="po_sb")
                    nc.vector.tensor_copy(po_sb, po[hh])
                    po_sbs.append(po_sb)
                return po_sbs

            def emit_zrow(po_sbs):
                # Z rows to partition 0 as f32r (regular tensor_copy handles
                # the partition offset; custom-DVE ops and the gpsimd queue
                # must NOT be involved here — the custom recip misreads
                # partition-offset inputs, and the gpsimd queue head-of-line
                # blocks on collective triggers).
                zrows = []
                for hh in range(2):
                    zr = zvp.tile([1, 512], F32R, name="zr")
                    with nc.allow_low_precision(reason="f32r Z"):
                        nc.vector.tensor_copy(zr, po_sbs[hh][DH : DH + 1, :])
                    zrows.append(zr)
                return zrows

            def emit_zb(zrows, hh):
                # broadcast raw Z across 64 partitions (PE rank-1 matmul;
                # the PE never stalls on the collective stream)
                zb = psx.tile([DH, 512], F32, name="zb", tag="psx")
                nc.tensor.matmul(zb, lhsT=ones_r, rhs=zrows[hh], start=True, stop=True)
                return zb

            def emit_recip(zb):
                # 1/Z on the partition-0-based PSUM tile (custom-DVE fast
                # reciprocal; ~5x faster than nc.vector.reciprocal)
                zbi = normp.tile([DH, 512], F32, name="zbi")
                nc.vector.reciprocal_approx_fast(out=zbi, in_=zb)
                return zbi

            def emit_mul(blk, po_sbs, zbi, hh):
                hp, qb = divmod(blk, 4)
                base = hh * DH
                with nc.allow_low_precision(reason="bf16 attention out"):
                    nc.vector.tensor_mul(
                        outt_sb[base : base + DH, hp, qb * 512 : (qb + 1) * 512],
                        po_sbs[hh][0:DH, :],
                        zbi,
                    )

            def emit_ag(blk):
                hp, qb = divmod(blk, 4)
                sl = slice(qb * 512, (qb + 1) * 512)
                nc.gpsimd.dma_start(out=cc_in[blk].ap(), in_=outt_sb[:, hp, sl])
                nc.gpsimd.collective_compute(
                    "AllGather",
                    mybir.AluOpType.bypass,
                    ins=[cc_in[blk].ap().opt()],
                    outs=[cc_out[blk].ap().opt()],
                    replica_groups=REPLICA_GROUPS,
                )
                nc.sync.dma_start(
                    out=ag_all[:, hp, qb, :, :],
                    in_=cc_out[blk].ap().rearrange("(c p) n -> p c n", p=128),
                )

            # --- interleaved output projection -------------------------------
            # piece qb: y^T[:, qb] = sum_hp wo[hp]^T @ ag[hp][qb] + bias.
            # Blocks alternate head pairs, so both AGs of a token quarter
            # land early and each piece is a single 8-chunk PSUM
            # accumulation per column block — no SBUF staging.
            def proj_tasks(qb):
                tasks = []
                ps_ref = {}

                def mk_mm(cb, hp, c0):
                    def f():
                        if hp == 0 and c0 == 0:
                            ps_ref[cb] = psx.tile(
                                [128, 512], F32, name="psy", tag="psx"
                            )
                        for c in range(c0, c0 + 2):
                            nc.tensor.matmul(
                                ps_ref[cb],
                                lhsT=wo_sb[:, hp * 4 + c, cb * 128 : (cb + 1) * 128],
                                rhs=ag_all[:, hp, qb, c, :],
                                start=(hp == 0 and c == 0),
                                stop=(hp == 1 and c == 3),
                            )

                    return f

                def mk_fin(cb):
                    def f():
                        qsl = slice(qb * 512, (qb + 1) * 512)
                        y_sb = yout.tile([128, 512], F32, name="y_sb")
                        nc.vector.tensor_scalar_add(
                            out=y_sb,
                            in0=ps_ref[cb],
                            scalar1=bias_sb[:, cb : cb + 1],
                        )
                        nc.sync.dma_start(
                            out=y[cb * 128 : (cb + 1) * 128, qsl], in_=y_sb
                        )

                    return f

                for cb in range(2):
                    for hp in range(2):
                        tasks.append(mk_mm(cb, hp, 0))
                        tasks.append(mk_mm(cb, hp, 2))
                    tasks.append(mk_fin(cb))
                return tasks

            # hp-major block order: piece qb needs AG(blk qb) (hp0, fired
            # block qb+1) and AG(blk 4+qb) (hp1, fired block 5+qb at kc==4).
            # Pieces 2 and 3 land in the tail, overlapping the final AG.
            proj_sched = {6: [0], 7: [1]}


            # one continuous software-pipelined stream over all 8 blocks:
            # attV lags dots/exp by one step; po drains to SBUF right after a
            # block's last attV; 1/Z, broadcast, and mul stages run over the
            # next block's early steps; the AllGather fires at kc==4.
            pend_attv = None  # (blk, kc, ex)
            po_cur = None
            posb_prev = None  # po_sbs of previous block
            zbi_prev = [None, None]
            task_q = []
            for step in range(NBLK * NKC):
                blk, kc = divmod(step, NKC)
                if kc == 0:
                    po_prev = po_cur
                    po_cur = [
                        pso.tile([DH + 1, 512], F32, name="ps_o") for _ in range(2)
                    ]
                    if blk == 1:
                        task_q += qk1_tasks()
                    task_q += [
                        t for q in proj_sched.get(blk, []) for t in proj_tasks(q)
                    ]
                ex = emit_dots(blk, kc)
                if pend_attv is not None:
                    pblk, pkc, pex = pend_attv
                    emit_attv(pblk, pkc, pex, po_cur if pblk == blk else po_prev)
                    if pkc == NKC - 1:
                        posb_cur = emit_posb(po_prev)
                pend_attv = (blk, kc, ex)
                if blk > 0:
                    if kc == 0:
                        posb_prev = posb_cur
                    elif kc == 1:
                        zrows_prev = emit_zrow(posb_prev)
                    elif kc == 2:
                        zb_prev = [
                            emit_zb(zrows_prev, 0),
                            emit_zb(zrows_prev, 1),
                        ]
                    elif kc == 3:
                        zbi_prev[0] = emit_recip(zb_prev[0])
                        zbi_prev[1] = emit_recip(zb_prev[1])
                    elif kc == 4:
                        emit_mul(blk - 1, posb_prev, zbi_prev[0], 0)
                        emit_mul(blk - 1, posb_prev, zbi_prev[1], 1)
                    elif kc == 5:
                        emit_ag(blk - 1)
                if blk == 0 and kc < TT - 8:
                    emit_v(kc + 8)
                if kc >= 6:
                    for _ in range(2):
                        if task_q:
                            task_q.pop(0)()
            # drain: the last block's normalization chain runs immediately
            # after its final attV so the last AllGather fires ASAP; proj
            # pieces 2 and 3 run after, overlapping the AG latencies.
            for t in task_q:
                t()
            pblk, pkc, pex = pend_attv
            emit_attv(pblk, pkc, pex, po_cur)
            po_sbs = emit_posb(po_cur)
            zrows = emit_zrow(po_sbs)
            zb0 = emit_zb(zrows, 0)
            zb1 = emit_zb(zrows, 1)
            zbi0 = emit_recip(zb0)
            zbi1 = emit_recip(zb1)
            emit_mul(NBLK - 1, po_sbs, zbi0, 0)
            emit_mul(NBLK - 1, po_sbs, zbi1, 1)
            emit_ag(NBLK - 1)
            for t in proj_tasks(2):
                t()
            for t in proj_tasks(3):
                t()

    nc.compile()
    return nc


_NC_CACHE = None


def _get_nc():
    global _NC_CACHE
    if _NC_CACHE is None:
        _NC_CACHE = build_nc()
    return _NC_CACHE


def _wo_perm(w_out):
    # chunk order [AG-hp0: r0..r3 -> w_out rows 256r..256r+128,
    #              AG-hp1: r0..r3 -> w_out rows 256r+128..256r+256]
    blocks = [w_out[256 * r : 256 * r + 128] for r in range(4)]
    blocks += [w_out[256 * r + 128 : 256 * r + 256] for r in range(4)]
    return np.concatenate(blocks, axis=0)


def _make_in_maps(x, w_qkv, w_out, b_out):
    wop = _wo_perm(w_out)
    in_maps = []
    for c in range(CORES):
        bi = c // GROUP_SIZE
        g = c % GROUP_SIZE
        cols = slice(g * CS, (g + 1) * CS)
        in_maps.append(
            {
                "xt": np.ascontiguousarray(x[bi].T).astype(NP_BF16),
                "wq": np.ascontiguousarray(w_qkv[:, cols]).astype(NP_BF16),
                "wk": np.ascontiguousarray(w_qkv[:, INNER:][:, cols]).astype(NP_BF16),
                "wv": np.ascontiguousarray(w_qkv[:, 2 * INNER:][:, cols]).astype(
                    NP_BF16
                ),
                "wo": np.ascontiguousarray(wop[:, cols]).astype(NP_BF16),
                "bo": np.ascontiguousarray(b_out[cols]),
            }
        )
    return in_maps


def _assemble(results):
    out = np.empty((B, N, DIM), dtype=np.float32)
    for c in range(CORES):
        bi = c // GROUP_SIZE
        g = c % GROUP_SIZE
        out[bi, :, g * CS : (g + 1) * CS] = results[c]["y"].T
    return out


def kernel(x, w_qkv, w_out, b_out, _trace=False, _trace_kwargs=None):
    x = np.asarray(x, dtype=np.float32)
    w_qkv = np.asarray(w_qkv, dtype=np.float32)
    w_out = np.asarray(w_out, dtype=np.float32)
    b_out = np.asarray(b_out, dtype=np.float32)
    nc = _get_nc()
    in_maps = _make_in_maps(x, w_qkv, w_out, b_out)
    res = run_bass_kernel_spmd(
        nc,
        in_maps,
        core_ids=list(range(CORES)),
        trace=_trace,
        **(_trace_kwargs or {}),
    )
    out = _assemble(res.results)
    if _trace:
        return out, res
    return out

